# revision 15
# baseline (speedup 1.0000x reference)
"""Trainium2 Bass kernel for nn_GumbelQuantize (vq_codebook).

Reference computation (per token row of N=1024 logits):
    g       = -log(-log(u))
    y_soft  = softmax((logits + g) / tau),  tau = 1
    ind     = argmax(y_soft)
    one_hot = y_hard + y_soft - stop_grad(y_soft)   # numerically == y_hard
    z_q     = one_hot @ embed                        # == embed[ind]
    qy      = softmax(logits)
    diff    = -log(qy * N + eps) / qy

Kernel algebra:
  * one_hot is exactly y_hard off-argmax (s - s == 0) and 1 +- 2^-23 at the
    argmax, so z_q = embed[ind]: an indirect-DMA row gather, no matmul.
  * argmax(y_soft) == argmax(logits - log(-log u)); softmax #1 is never
    materialized.  Z is computed with the same op sequence as the reference
    (T = ln u; S = ln(-T); Z = L - S) to minimize tie-flip risk.
  * qy needs exp(logits) once:  E = exp(L) with fused row-sum (accum_out).
    diff = -ln(E * (N/se) + eps) * se * exp(-L), using exp(-L) from ACT
    instead of an iterative DVE divide.

Sharding: data-parallel over B -- core i gets batch row i (S=2048 tokens),
the [N, D] codebook is replicated.  No inter-core communication.
"""

import numpy as np

import concourse.bass as bass
import concourse.bacc as bacc
import concourse.tile as tile
from concourse import mybir
from concourse.bass_utils import run_bass_kernel_spmd

B, S, N, D = 8, 2048, 1024, 512
P = 128
F32 = mybir.dt.float32
EPS = 1e-10

# Set by test harness to collect an NTFF profile; kernel() stores results here.
TRACE = False
LAST_RESULTS = None


class _Bacc(bacc.Bacc):
    """Bacc that forces the combined Exp+Ln activation table.

    The default table-selection pass alternates exp-only / ln-only sets,
    inserting a ~1.3us LoadActFuncSet per function switch (46 loads, ~59us
    of ACT time here).  Keeping only sets that contain BOTH Exp and Ln
    (list indices preserved -- they are the act_func_set_id walrus reads)
    makes every activation resolve to one set, so the fixpoint hoists a
    single load.
    """

    def insert_act_table_loads(self):
        import bass_rust as _bass_rust
        from concourse.hw_specs import get_activation_tables
        from concourse import mybir as mb

        has_activation = any(
            isinstance(i, mb.InstActivation)
            for b in self.main_func.blocks
            for i in b.instructions
        )
        if not has_activation:
            return
        Exp = mb.ActivationFunctionType.Exp
        Ln = mb.ActivationFunctionType.Ln
        tables = [
            (name, (s if (Exp in s and Ln in s) else set()))
            for name, s in get_activation_tables(self.m.arch).items()
        ]
        _bass_rust.insert_act_table_loads(self, tables)


def build_kernel(s_per_core: int = S) -> bass.Bass:
    nt = s_per_core // P
    nc = _Bacc()

    logits_h = nc.declare_dram_parameter("logits", [s_per_core, N], F32, isOutput=False)
    u_h = nc.declare_dram_parameter("u", [s_per_core, N], F32, isOutput=False)
    embed_h = nc.declare_dram_parameter("embed", [N, D], F32, isOutput=False)
    zq_h = nc.declare_dram_parameter("z_q", [s_per_core, D], F32, isOutput=True)
    diff_h = nc.declare_dram_parameter("diff", [s_per_core, N], F32, isOutput=True)
    ind_h = nc.declare_dram_parameter("ind", [s_per_core], mybir.dt.int32, isOutput=True)

    Exp = mybir.ActivationFunctionType.Exp
    Ln = mybir.ActivationFunctionType.Ln
    mult = mybir.AluOpType.mult

    with tile.TileContext(nc) as tc:
        with (
            tc.tile_pool(name="work", bufs=4) as wp,
            tc.tile_pool(name="small", bufs=4) as sp,
            tc.tile_pool(name="persist", bufs=1) as pp,
        ):
            ind_all = pp.tile([P, nt], mybir.dt.uint32)
            eps_t = pp.tile([P, 1], F32)
            nc.gpsimd.memset(eps_t[:], EPS)
            prev = None
            for t in range(nt + 1):
                # diff path of tile t-1 first: its inputs are a full
                # iteration old, so ACT/DVE never stall on the
                # recip -> G -> DF cross-engine chain.
                if prev is not None:
                    G = wp.tile([P, N], F32, tag="G")
                    nc.scalar.activation(
                        G[:], prev["E"][:], Ln,
                        scale=prev["rse_n"][:, :1], bias=eps_t[:, :1],
                    )
                    DF = wp.tile([P, N], F32, tag="DF")
                    nc.vector.scalar_tensor_tensor(
                        out=DF[:], in0=G[:], scalar=prev["nse"][:, :1],
                        in1=prev["EN"][:], op0=mult, op1=mult,
                    )
                    nc.sync.dma_start(out=diff_h[prev["rows"], :], in_=DF[:])
                if t >= nt:
                    break
                rows = slice(t * P, (t + 1) * P)
                L = wp.tile([P, N], F32, tag="L")
                U = wp.tile([P, N], F32, tag="U")
                nc.sync.dma_start(out=L[:], in_=logits_h[rows, :])
                nc.sync.dma_start(out=U[:], in_=u_h[rows, :])

                # E = exp(L), se = row-sum(E); EN = exp(-L)
                E = wp.tile([P, N], F32, tag="E")
                se = sp.tile([P, 1], F32, tag="se")
                nc.scalar.activation(E[:], L[:], Exp, accum_out=se[:])
                EN = wp.tile([P, N], F32, tag="EN")
                nc.scalar.activation(EN[:], L[:], Exp, scale=-1.0)

                # Z = L - ln(-ln(U))   (gumbel-perturbed logits)
                T_ = wp.tile([P, N], F32, tag="T")
                nc.scalar.activation(T_[:], U[:], Ln)
                Sg = wp.tile([P, N], F32, tag="S")
                nc.scalar.activation(Sg[:], T_[:], Ln, scale=-1.0)
                Z = wp.tile([P, N], F32, tag="Z")
                nc.vector.tensor_sub(Z[:], L[:], Sg[:])

                # row argmax of Z
                m8 = sp.tile([P, 8], F32, tag="m8")
                nc.vector.max(m8[:], Z[:])
                i8 = sp.tile([P, 8], mybir.dt.uint32, tag="i8")
                nc.vector.max_index(i8[:], m8[:], Z[:])
                nc.vector.tensor_copy(ind_all[:, t : t + 1], i8[:, :1])

                # z_q rows: gather embed[ind] via indirect DMA
                zq = wp.tile([P, D], F32, tag="zq")
                nc.gpsimd.indirect_dma_start(
                    out=zq[:],
                    out_offset=None,
                    in_=embed_h[:],
                    in_offset=bass.IndirectOffsetOnAxis(ap=i8[:, :1], axis=0),
                )
                nc.sync.dma_start(out=zq_h[rows, :], in_=zq[:])

                # per-row scalars for the diff path (consumed next iteration)
                se_n = sp.tile([P, 1], F32, tag="se_n")  # se/N
                rse_n = sp.tile([P, 1], F32, tag="rse_n")  # N/se
                nse = sp.tile([P, 1], F32, tag="nse")  # -se
                nc.vector.tensor_scalar_mul(se_n[:], se[:], 1.0 / N)
                nc.vector.reciprocal(rse_n[:], se_n[:])
                nc.vector.tensor_scalar_mul(nse[:], se[:], -1.0)
                prev = {"E": E, "EN": EN, "rse_n": rse_n, "nse": nse, "rows": rows}

            # all token indices in one strided store: ind[t*P + p] = ind_all[p, t]
            ind_view = ind_h[:].rearrange("(t p) -> p t", p=P)
            nc.sync.dma_start(out=ind_view, in_=ind_all[:].bitcast(mybir.dt.int32))
    nc.finalize()  # Bacc: alloc regs + split multi-waits into event semaphores
    return nc


def kernel(logits: np.ndarray, u: np.ndarray, embed: np.ndarray):
    global LAST_RESULTS
    logits = np.ascontiguousarray(np.asarray(logits, dtype=np.float32))
    u = np.ascontiguousarray(np.asarray(u, dtype=np.float32))
    embed = np.ascontiguousarray(np.asarray(embed, dtype=np.float32))

    nc = build_kernel()
    in_maps = [
        {"logits": logits[i], "u": u[i], "embed": embed} for i in range(B)
    ]
    res = run_bass_kernel_spmd(nc, in_maps, core_ids=list(range(B)), trace=TRACE)
    LAST_RESULTS = res
    z_q = np.stack([r["z_q"] for r in res.results])
    diff = np.stack([r["diff"] for r in res.results])
    ind = np.stack([r["ind"] for r in res.results])
    return z_q, diff, ind
